# revision 1
# baseline (speedup 1.0000x reference)
"""Trainium2 Bass kernel: gumbel-softmax-argmax embedding lookup (end-to-end).

Reference math (nn_End2End_49495203119139):
    hot  = argmax_V(softmax((logits + gumbel)/tau))       == argmax_V(logits+gumbel)
    row  = grid_sample-nearest index map of hot            == ROWMAP[hot]  (LUT)
    tok_emb = W[row][:, col_map]   with col_map == arange(E)  (verified at runtime)
    inputs_embeds = tok_emb * mask
    psg_roll = roll(psg_ids, 1, axis=1); psg_roll[:,0] = 1
    extr  = (1 - mask[:, ::-1]) * psg_roll
    trunc = rotate_right(extr, shifts) with shifts = mask.sum(-1)   (per row)
    flag  = cumsum(trunc != 0, -1) > 0
    out   = inputs_embeds + where(flag, W[trunc], 0)

Sharding: data-parallel over batch. B=16 over 8 cores -> 2 batch rows
(= 2 token tiles of 128) per core; the embedding table is replicated.

Per-core device plan (memory-bound part = streaming logits+gumbel, 66 MB,
~184 us HBM floor at ~358 GB/s per core):
  - for each token tile (128 tokens on partitions) and each vocab chunk
    [128 x 2008]: HWDGE-load the logits chunk, then add the gumbel chunk
    with one SWDGE inline-accumulate DMA (CCE add; descriptors must stay
    <= 2048 elements — larger accumulates crash the device).
    DVE `max` finds the chunk max, `max_index` the first within-chunk
    argmax position (ties resolve to the lowest index, matching argmax).
  - chunk winner (lowest chunk attaining the global max) + within-chunk
    index give `hot`; ROWMAP and W rows come via indirect DMA gathers.
  - the passage branch is pure index arithmetic on [128,1] tiles: the
    reverse/roll/rotate are folded into gather indices modulo L, the
    mask-sum and cumsum are 0/1 matmuls against ones/triangular matrices
    (exact in any PE precision).
Predicted 212.4 us/core by the TimelineSim cost model (DMA engines busy
190 us of that, i.e. ~97% of the 66MB/358GB/s floor); a hardware
min-slope measurement of the 4016-chunk variant gave ~204 us.
"""

import numpy as np

B = 16
L = 128
V = 32128
E = 768
N_CORES = 8
B_LOC = B // N_CORES          # batch rows per core
CH = 2008                     # vocab chunk (free dim) per streamed tile;
                              # <= 2048 so a gumbel chunk is ONE CCE-add DMA
NCH = V // CH                 # 16 chunks
NEG = -3.0e38


def _build(nc_mod, dims=None, body_reps=1):
    """Build the per-core Bass module. dims allows small smoke-test builds;
    body_reps>1 repeats the whole body (for slope-based benchmarking)."""
    import concourse.tile as tile
    from concourse import bass, mybir
    from concourse.bass import IndirectOffsetOnAxis

    d = dims or {}
    v = d.get("V", V)
    e = d.get("E", E)
    ch = d.get("CH", CH)
    nch = v // ch
    b_loc = d.get("B_LOC", B_LOC)
    rows = b_loc * L
    lbufs = d.get("LBUFS", 8)
    skip_tail = d.get("SKIP_TAIL", False)
    skip_accum = d.get("SKIP_ACCUM", False)
    skip_maxidx = d.get("SKIP_MAXIDX", False)
    tail_after_each = d.get("TAIL_AFTER_EACH", False)
    # how logits+gumbel are summed: "accum" = DMA CCE inline add (SWDGE),
    # "dve" = plain loads + DVE adds, "split" = adds alternate DVE/GpSimd
    add_mode = d.get("ADD_MODE", "accum")
    # chunk spans (lo, size); SMALL_LAST splits the final chunk so the
    # post-last-DMA DVE chain (max+max_index of the last chunk) is short
    spans = [(c * ch, ch) for c in range(nch)]
    if d.get("SMALL_LAST", False) and ch >= 1024:
        lo_last, sz = spans.pop()
        spans.append((lo_last, sz - 502))
        spans.append((lo_last + sz - 502, 502))
    nsp = len(spans)
    max_eng = d.get("MAX_ENG", "dve")
    smalls_on_act = d.get("SMALLS_ON_ACT", False)

    nc = nc_mod
    f32 = mybir.dt.float32
    i32 = mybir.dt.int32
    u32 = mybir.dt.uint32
    Op = mybir.AluOpType
    AX = mybir.AxisListType

    two_tables = d.get("TWO_TABLES", False)

    logits_h = nc.dram_tensor("logits", [rows, v], f32, kind="ExternalInput")
    gumbel_h = nc.dram_tensor("gumbel", [rows, v], f32, kind="ExternalInput")
    mask_h = nc.dram_tensor("mask", [rows, 1], i32, kind="ExternalInput")
    psg_h = nc.dram_tensor("psg", [rows, 1], i32, kind="ExternalInput")
    wte_h = nc.dram_tensor("wte", [v, e], f32, kind="ExternalInput")
    # the token branch reads W[:, col_map]; col_map is the identity here, so
    # both branches normally share one table (TWO_TABLES is a safety fallback)
    wtok_h = nc.dram_tensor("wte_tok", [v, e], f32, kind="ExternalInput") if two_tables else wte_h
    rowmap_h = nc.dram_tensor("rowmap", [v, 1], i32, kind="ExternalInput")
    tri_h = nc.dram_tensor("tri", [L, L], f32, kind="ExternalInput")
    out_h = nc.dram_tensor("out", [rows, e], f32, kind="ExternalOutput")
    # tiny passthrough pair so a benchmark can chain executions back-to-back
    chain_h = nc.dram_tensor("chain", [L, 8], f32, kind="ExternalInput")
    chain_o = nc.dram_tensor("chain_out", [L, 8], f32, kind="ExternalOutput")

    with tile.TileContext(nc) as tc:
        with (
            tc.tile_pool(name="lpool", bufs=lbufs) as lpool,
            tc.tile_pool(name="stats", bufs=d.get("SBUFS", 2)) as stats,
            tc.tile_pool(name="small", bufs=d.get("SBUFS", 2)) as small,
            tc.tile_pool(name="emb", bufs=d.get("SBUFS", 2)) as emb,
            tc.tile_pool(name="consts", bufs=1) as consts,
            tc.tile_pool(name="psum", bufs=2, space="PSUM") as psum,
        ):
            # ---- benchmark chain passthrough ----
            cht = consts.tile([L, 8], f32)
            nc.scalar.dma_start(out=cht[:], in_=chain_h[:])
            nc.scalar.dma_start(out=chain_o[:], in_=cht[:])

            # ---- per-core constants (built once) ----
            ones_mat = consts.tile([L, L], f32)
            nc.vector.memset(ones_mat[:], 1.0)
            tri_sb = consts.tile([L, L], f32)
            nc.scalar.dma_start(out=tri_sb[:], in_=tri_h[:])

            iota_p_i = consts.tile([L, 1], i32)
            nc.gpsimd.iota(iota_p_i[:], pattern=[[1, 1]], base=0, channel_multiplier=1)
            iota_p = consts.tile([L, 1], f32)
            nc.vector.tensor_copy(out=iota_p[:], in_=iota_p_i[:])

            iota8_i = consts.tile([L, nsp], i32)
            nc.gpsimd.iota(iota8_i[:], pattern=[[1, nsp]], base=0, channel_multiplier=0)
            iota8 = consts.tile([L, nsp], f32)
            nc.vector.tensor_copy(out=iota8[:], in_=iota8_i[:])
            # c8rev[c] = nsp - c  (used to pick the LOWEST chunk that attains the max)
            c8rev = consts.tile([L, nsp], f32)
            nc.vector.tensor_scalar(c8rev[:], iota8[:], -1.0, float(nsp), op0=Op.mult, op1=Op.add)
            # per-chunk start offsets (hot = bases[c*] + within-chunk index)
            bases = consts.tile([L, nsp], f32)
            nc.vector.tensor_scalar(bases[:], iota8[:], float(ch), None, op0=Op.mult)
            for ci, (lo_c, _sz) in enumerate(spans):
                if lo_c != ci * ch:
                    nc.vector.memset(bases[:, ci:ci + 1], float(lo_c))

            def psg_phase(t):
                """Everything that does not depend on the streamed logits:
                mask/psg index arithmetic, flag, psg-embedding gather."""
                tok = slice(t * L, (t + 1) * L)
                mask_i = small.tile([L, 1], i32, tag="mask_i")
                nc.scalar.dma_start(out=mask_i[:], in_=mask_h[tok, :])
                mask_f = small.tile([L, 1], f32, tag="mask_f")
                nc.vector.tensor_copy(out=mask_f[:], in_=mask_i[:])

                # s (broadcast to all partitions) = sum(mask) via ones matmul
                s_ps = psum.tile([L, 1], f32, tag="s_ps")
                nc.tensor.matmul(out=s_ps[:], lhsT=ones_mat[:], rhs=mask_f[:], start=True, stop=True)
                s_bc = small.tile([L, 1], f32, tag="s_bc")
                nc.vector.tensor_copy(out=s_bc[:], in_=s_ps[:])

                def mod_l(x_ap, lo_fix=True, hi_fix=True, tagp=""):
                    # x <- x mod L for x in (-L, 2L)
                    if hi_fix:
                        ge = small.tile([L, 1], f32, tag="ge" + tagp)
                        nc.vector.tensor_scalar(ge[:], x_ap, float(L), None, op0=Op.is_ge)
                        nc.vector.scalar_tensor_tensor(
                            out=x_ap, in0=ge[:], scalar=-float(L), in1=x_ap, op0=Op.mult, op1=Op.add)
                    if lo_fix:
                        lt_ = small.tile([L, 1], f32, tag="lt" + tagp)
                        nc.vector.tensor_scalar(lt_[:], x_ap, 0.0, None, op0=Op.is_lt)
                        nc.vector.scalar_tensor_tensor(
                            out=x_ap, in0=lt_[:], scalar=float(L), in1=x_ap, op0=Op.mult, op1=Op.add)

                # fidx = (L-1 + s - l) mod L   (flipped-mask gather index)
                fidx = small.tile([L, 1], f32, tag="fidx")
                nc.vector.scalar_tensor_tensor(
                    out=fidx[:], in0=s_bc[:], scalar=float(L - 1), in1=iota_p[:],
                    op0=Op.add, op1=Op.subtract)
                mod_l(fidx[:], lo_fix=False, tagp="f")

                # pidx = (L-1 - s + l) mod L   (rolled-psg gather index)
                pidx = small.tile([L, 1], f32, tag="pidx")
                nc.vector.scalar_tensor_tensor(
                    out=pidx[:], in0=s_bc[:], scalar=-1.0, in1=iota_p[:],
                    op0=Op.mult, op1=Op.add)
                nc.vector.tensor_scalar(pidx[:], pidx[:], float(L - 1), None, op0=Op.add)
                mod_l(pidx[:], tagp="p")

                # k = (l - s) mod L ; BOS position is k == 0
                kk = small.tile([L, 1], f32, tag="kk")
                nc.vector.scalar_tensor_tensor(
                    out=kk[:], in0=s_bc[:], scalar=-1.0, in1=iota_p[:],
                    op0=Op.mult, op1=Op.add)
                mod_l(kk[:], hi_fix=False, tagp="k")
                bos = small.tile([L, 1], f32, tag="bos")
                nc.vector.tensor_scalar(bos[:], kk[:], 0.0, None, op0=Op.is_equal)

                # gather mask[fidx] and psg[pidx] (within this batch row)
                fr_i = small.tile([L, 1], i32, tag="fr_i")
                nc.vector.tensor_scalar(fidx[:], fidx[:], float(t * L), None, op0=Op.add)
                nc.vector.tensor_copy(out=fr_i[:], in_=fidx[:])
                mrev = small.tile([L, 1], i32, tag="mrev")
                nc.gpsimd.indirect_dma_start(
                    out=mrev[:], out_offset=None, in_=mask_h[:],
                    in_offset=IndirectOffsetOnAxis(ap=fr_i[:, 0:1], axis=0),
                )
                pr_i = small.tile([L, 1], i32, tag="pr_i")
                nc.vector.tensor_scalar(pidx[:], pidx[:], float(t * L), None, op0=Op.add)
                nc.vector.tensor_copy(out=pr_i[:], in_=pidx[:])
                prot = small.tile([L, 1], i32, tag="prot")
                nc.gpsimd.indirect_dma_start(
                    out=prot[:], out_offset=None, in_=psg_h[:],
                    in_offset=IndirectOffsetOnAxis(ap=pr_i[:, 0:1], axis=0),
                )

                # f_rot = 1 - mask[fidx]
                mrev_f = small.tile([L, 1], f32, tag="mrev_f")
                nc.vector.tensor_copy(out=mrev_f[:], in_=mrev[:])
                frot = small.tile([L, 1], f32, tag="frot")
                nc.vector.tensor_scalar(frot[:], mrev_f[:], -1.0, 1.0, op0=Op.mult, op1=Op.add)
                # psg_rot = bos ? 1 : psg[pidx]
                prot_f = small.tile([L, 1], f32, tag="prot_f")
                nc.vector.tensor_copy(out=prot_f[:], in_=prot[:])
                nbos = small.tile([L, 1], f32, tag="nbos")
                nc.vector.tensor_scalar(nbos[:], bos[:], -1.0, 1.0, op0=Op.mult, op1=Op.add)
                nc.vector.tensor_tensor(out=prot_f[:], in0=prot_f[:], in1=nbos[:], op=Op.mult)
                nc.vector.tensor_tensor(out=prot_f[:], in0=prot_f[:], in1=bos[:], op=Op.add)
                # trunc = f_rot * psg_rot
                trunc = small.tile([L, 1], f32, tag="trunc")
                nc.vector.tensor_tensor(out=trunc[:], in0=frot[:], in1=prot_f[:], op=Op.mult)

                # flag = cumsum(trunc != 0) > 0 via triangular matmul
                nz = small.tile([L, 1], f32, tag="nz")
                nc.vector.tensor_scalar(nz[:], trunc[:], 0.0, None, op0=Op.not_equal)
                cnt_ps = psum.tile([L, 1], f32, tag="cnt_ps")
                nc.tensor.matmul(out=cnt_ps[:], lhsT=tri_sb[:], rhs=nz[:], start=True, stop=True)
                flag = small.tile([L, 1], f32, tag="flag")
                nc.vector.tensor_scalar(flag[:], cnt_ps[:], 0.0, None, op0=Op.is_gt)

                trunc_i = small.tile([L, 1], i32, tag="trunc_i")
                nc.vector.tensor_copy(out=trunc_i[:], in_=trunc[:])
                psgemb = emb.tile([L, e], f32, tag="psgemb")
                nc.gpsimd.indirect_dma_start(
                    out=psgemb[:], out_offset=None, in_=wte_h[:],
                    in_offset=IndirectOffsetOnAxis(ap=trunc_i[:, 0:1], axis=0),
                )
                return mask_f, flag, psgemb

            def stream_phase(t):
                """DMA-bound pass over the vocab: per chunk, load logits,
                accumulate gumbel in the DMA datapath, track max + argmax."""
                tok = slice(t * L, (t + 1) * L)
                m_all = stats.tile([L, nsp], f32, tag="m_all")
                idx_all = stats.tile([L, nsp], f32, tag="idx_all")
                for c, (lo, csz) in enumerate(spans):
                    lt = lpool.tile([L, ch], f32, tag="lt")
                    ldeng = nc.scalar if (d.get("DUAL_HWDGE", True) and c % 2) else nc.sync
                    ldeng.dma_start(out=lt[:, 0:csz], in_=logits_h[tok, lo:lo + csz])
                    if add_mode == "accum":
                        # s = logits + gumbel via DMA CCE inline add;
                        # descriptors must stay <= 2048 elements each.
                        half = ch // 2
                        if not skip_accum and ch <= 2048:
                            nc.gpsimd.dma_start(
                                out=lt[:, 0:csz], in_=gumbel_h[tok, lo:lo + csz],
                                accum_op=Op.add)
                        elif not skip_accum:
                            if d.get("ACCUM3D", False):
                                gsrc = gumbel_h[tok, lo:lo + ch].rearrange(
                                    "p (a b) -> p a b", b=half)
                                ldst = lt[:].rearrange("p (a b) -> p a b", b=half)
                                nc.gpsimd.dma_start(out=ldst, in_=gsrc, accum_op=Op.add)
                            else:
                                nc.gpsimd.dma_start(
                                    out=lt[:, 0:half], in_=gumbel_h[tok, lo:lo + half],
                                    accum_op=Op.add)
                                nc.gpsimd.dma_start(
                                    out=lt[:, half:ch], in_=gumbel_h[tok, lo + half:lo + ch],
                                    accum_op=Op.add)
                    elif add_mode == "hybrid":
                        # half the gumbel chunk via SWDGE inline-add DMA,
                        # half via HWDGE load + DVE add: balances Pool.SEQ
                        # descriptor emission against DVE cycles.
                        half = ch // 2
                        nc.gpsimd.dma_start(
                            out=lt[:, 0:half], in_=gumbel_h[tok, lo:lo + half],
                            accum_op=Op.add)
                        gt = lpool.tile([L, half], f32, tag="gt")
                        nc.sync.dma_start(out=gt[:], in_=gumbel_h[tok, lo + half:lo + ch])
                        nc.vector.tensor_tensor(out=lt[:, half:ch], in0=lt[:, half:ch], in1=gt[:], op=Op.add)
                    else:
                        gt = lpool.tile([L, ch], f32, tag="gt")
                        nc.sync.dma_start(out=gt[:], in_=gumbel_h[tok, lo:lo + ch])
                        eng = nc.vector if (add_mode == "dve" or c % 2 == 0) else nc.gpsimd
                        eng.tensor_tensor(out=lt[:], in0=lt[:], in1=gt[:], op=Op.add)
                    # chunk max + within-chunk argmax (first occurrence);
                    # the column copies go to the otherwise-idle ACT engine
                    mx8 = small.tile([L, 8], f32, tag="mx8")
                    nc.vector.max(out=mx8[:], in_=lt[:, 0:csz])
                    if smalls_on_act:
                        nc.scalar.copy(out=m_all[:, c:c + 1], in_=mx8[:, 0:1])
                    else:
                        nc.vector.tensor_copy(out=m_all[:, c:c + 1], in_=mx8[:, 0:1])
                    mi8 = small.tile([L, 8], u32, tag="mi8")
                    if not skip_maxidx:
                        nc.vector.max_index(out=mi8[:], in_max=mx8[:], in_values=lt[:, 0:csz])
                    else:
                        nc.vector.memset(mi8[:], 0)
                    if smalls_on_act:
                        nc.scalar.copy(out=idx_all[:, c:c + 1], in_=mi8[:, 0:1])
                    else:
                        nc.vector.tensor_copy(out=idx_all[:, c:c + 1], in_=mi8[:, 0:1])
                return m_all, idx_all

            def tail_phase(t, m_all, idx_all, mask_f, flag, psgemb):
                tok = slice(t * L, (t + 1) * L)
                # global max + first chunk attaining it
                gmax = small.tile([L, 1], f32, tag="gmax")
                nc.vector.reduce_max(out=gmax[:], in_=m_all[:], axis=AX.X)
                sel8 = small.tile([L, nsp], f32, tag="sel8")
                nc.vector.scalar_tensor_tensor(
                    out=sel8[:], in0=m_all[:], scalar=gmax[:, 0:1], in1=c8rev[:],
                    op0=Op.is_ge, op1=Op.mult)
                cmax = small.tile([L, 1], f32, tag="cmax")
                nc.vector.reduce_max(out=cmax[:], in_=sel8[:], axis=AX.X)
                cstar = small.tile([L, 1], f32, tag="cstar")
                nc.vector.tensor_scalar(cstar[:], cmax[:], -1.0, float(nsp), op0=Op.mult, op1=Op.add)
                # winning chunk's within-chunk index and base offset
                junk8 = small.tile([L, nsp], f32, tag="junk8")
                nc.vector.scalar_tensor_tensor(
                    out=junk8[:], in0=iota8[:], scalar=cstar[:, 0:1], in1=idx_all[:],
                    op0=Op.is_equal, op1=Op.mult)
                mi_sel = small.tile([L, 1], f32, tag="mi_sel")
                nc.vector.reduce_max(out=mi_sel[:], in_=junk8[:], axis=AX.X)
                junk8b = small.tile([L, nsp], f32, tag="junk8b")
                nc.vector.scalar_tensor_tensor(
                    out=junk8b[:], in0=iota8[:], scalar=cstar[:, 0:1], in1=bases[:],
                    op0=Op.is_equal, op1=Op.mult)
                base_sel = small.tile([L, 1], f32, tag="base_sel")
                nc.vector.reduce_max(out=base_sel[:], in_=junk8b[:], axis=AX.X)
                hot_f = small.tile([L, 1], f32, tag="hot_f")
                nc.vector.tensor_tensor(out=hot_f[:], in0=base_sel[:], in1=mi_sel[:], op=Op.add)
                hot_i = small.tile([L, 1], i32, tag="hot_i")
                nc.vector.tensor_copy(out=hot_i[:], in_=hot_f[:])
                if d.get("MERGED_TAIL", False):
                    return hot_i

                # hot -> vocab row (grid_sample LUT), -> token embeddings
                rowidx = small.tile([L, 1], i32, tag="rowidx")
                nc.gpsimd.indirect_dma_start(
                    out=rowidx[:], out_offset=None, in_=rowmap_h[:],
                    in_offset=IndirectOffsetOnAxis(ap=hot_i[:, 0:1], axis=0),
                )
                tokemb = emb.tile([L, e], f32, tag="tokemb")
                nc.gpsimd.indirect_dma_start(
                    out=tokemb[:], out_offset=None, in_=wtok_h[:],
                    in_offset=IndirectOffsetOnAxis(ap=rowidx[:, 0:1], axis=0),
                )

                # combine + store
                p1 = emb.tile([L, e], f32, tag="p1")
                nc.vector.tensor_scalar(p1[:], tokemb[:], mask_f[:, 0:1], None, op0=Op.mult)
                outt = emb.tile([L, e], f32, tag="outt")
                nc.vector.scalar_tensor_tensor(
                    out=outt[:], in0=psgemb[:], scalar=flag[:, 0:1], in1=p1[:],
                    op0=Op.mult, op1=Op.add)
                nc.sync.dma_start(out=out_h[tok, :], in_=outt[:])

            for _ in range(body_reps):
                if skip_tail:
                    for t in range(b_loc):
                        m_all, idx_all = stream_phase(t)
                        tok = slice(t * L, (t + 1) * L)
                        dummy = emb.tile([L, e], f32, tag="outt")
                        nc.vector.tensor_scalar(dummy[:], m_all[:, 0:1].to_broadcast([L, e]), 1.0, None, op0=Op.mult)
                        nc.sync.dma_start(out=out_h[tok, :], in_=dummy[:])
                    continue
                psg_state = [psg_phase(t) for t in range(b_loc)]
                if tail_after_each:
                    for t in range(b_loc):
                        m_all, idx_all = stream_phase(t)
                        tail_phase(t, m_all, idx_all, *psg_state[t])
                elif d.get("MERGED_TAIL", False) and b_loc == 2:
                    streams = [stream_phase(t) for t in range(b_loc)]
                    hots = [tail_phase(t, *streams[t], *psg_state[t]) for t in range(b_loc)]
                    # one multi-index gather for both tiles: rowmap then W rows
                    hot2 = small.tile([L, 2], i32, tag="hot2")
                    nc.vector.tensor_copy(out=hot2[:, 0:1], in_=hots[0][:])
                    nc.vector.tensor_copy(out=hot2[:, 1:2], in_=hots[1][:])
                    ridx2 = small.tile([L, 2], i32, tag="ridx2")
                    nc.gpsimd.indirect_dma_start(
                        out=ridx2[:], out_offset=None, in_=rowmap_h[:],
                        in_offset=IndirectOffsetOnAxis(ap=hot2[:, 0:2], axis=0))
                    tok2 = emb.tile([L, 2 * e], f32, tag="tok2")
                    nc.gpsimd.indirect_dma_start(
                        out=tok2[:], out_offset=None, in_=wtok_h[:],
                        in_offset=IndirectOffsetOnAxis(ap=ridx2[:, 0:2], axis=0))
                    for t in range(b_loc):
                        mask_f, flag, psgemb = psg_state[t]
                        p1 = emb.tile([L, e], f32, tag="p1")
                        nc.vector.tensor_scalar(p1[:], tok2[:, t * e:(t + 1) * e], mask_f[:, 0:1], None, op0=Op.mult)
                        outt = emb.tile([L, e], f32, tag="outt")
                        nc.vector.scalar_tensor_tensor(
                            out=outt[:], in0=psgemb[:], scalar=flag[:, 0:1], in1=p1[:],
                            op0=Op.mult, op1=Op.add)
                        nc.sync.dma_start(out=out_h[t * L:(t + 1) * L, :], in_=outt[:])
                else:
                    streams = [stream_phase(t) for t in range(b_loc)]
                    for t in range(b_loc):
                        tail_phase(t, *streams[t], *psg_state[t])

    return nc


_BUILD_CACHE = {}


def _get_module(dims_key=None, dims=None, body_reps=1):
    key = (dims_key, body_reps)
    if key not in _BUILD_CACHE:
        import concourse.bacc as bacc

        nc = bacc.Bacc("TRN2", target_bir_lowering=False, debug=False)
        _build(nc, dims, body_reps=body_reps)
        nc.compile()
        _BUILD_CACHE[key] = nc
    return _BUILD_CACHE[key]


_ROWMAP_CACHE = {}


def _nearest_maps():
    """Replicate the reference's f32 grid_sample-nearest index maps with jnp
    on the same backend the reference runs on (bit-exact by construction)."""
    if "maps" not in _ROWMAP_CACHE:
        import jax.numpy as jnp

        def nearest(size):
            lin = jnp.linspace(-1.0, 1.0, size)
            ix = ((lin + 1.0) * size - 1.0) / 2.0
            return np.asarray(jnp.clip(jnp.round(ix), 0, size - 1).astype(jnp.int32))

        _ROWMAP_CACHE["maps"] = (nearest(V), nearest(E))
    return _ROWMAP_CACHE["maps"]


_TRI = None

# test/dev hooks: set TRACE=True before calling kernel() to capture an NTFF
# profile; the BassKernelResults of the last run is stored in LAST_RESULT.
TRACE = False
LAST_RESULT = None


def kernel(logits, rwrt_attention_mask, psg_input_ids, word_embeddings, gumbel_noise):
    from concourse.bass_utils import run_bass_kernel_spmd

    global _TRI
    logits = np.ascontiguousarray(np.asarray(logits, dtype=np.float32))
    gumbel = np.ascontiguousarray(np.asarray(gumbel_noise, dtype=np.float32))
    mask = np.ascontiguousarray(np.asarray(rwrt_attention_mask, dtype=np.int32))
    psg = np.ascontiguousarray(np.asarray(psg_input_ids, dtype=np.int32))
    wte = np.ascontiguousarray(np.asarray(word_embeddings, dtype=np.float32))

    rowmap, colmap = _nearest_maps()
    col_identity = bool(np.array_equal(colmap, np.arange(E, dtype=np.int32)))
    rowmap2 = rowmap.reshape(V, 1)
    if _TRI is None:
        _TRI = np.ascontiguousarray(np.triu(np.ones((L, L), dtype=np.float32)))

    if col_identity:
        nc = _get_module()
    else:
        # safety fallback (never taken in this environment): bake the column
        # permutation into a separate token-branch table
        nc = _get_module(dims_key="two_tables", dims={"TWO_TABLES": True})
        wte_tok = np.ascontiguousarray(wte[:, colmap])

    in_maps = []
    for m in range(N_CORES):
        sl = slice(m * B_LOC, (m + 1) * B_LOC)
        im = {
            "logits": logits[sl].reshape(B_LOC * L, V),
            "gumbel": gumbel[sl].reshape(B_LOC * L, V),
            "mask": mask[sl].reshape(B_LOC * L, 1),
            "psg": psg[sl].reshape(B_LOC * L, 1),
            "wte": wte,
            "rowmap": rowmap2,
            "tri": _TRI,
            "chain": np.zeros((L, 8), np.float32),
        }
        if not col_identity:
            im["wte_tok"] = wte_tok
        in_maps.append(im)

    global LAST_RESULT
    try:
        LAST_RESULT = run_bass_kernel_spmd(nc, in_maps, list(range(N_CORES)), trace=TRACE)
    except Exception:
        # the axon-relayed device occasionally reports a transient
        # NRT_EXEC_UNIT_UNRECOVERABLE on the first execution after long
        # sessions; a straight re-run recovers it
        import time as _time

        _time.sleep(2.0)
        LAST_RESULT = run_bass_kernel_spmd(nc, in_maps, list(range(N_CORES)), trace=TRACE)
    res = LAST_RESULT.results
    out = np.concatenate(
        [res[m]["out"].reshape(B_LOC, L, E) for m in range(N_CORES)], axis=0
    )
    return out



# revision 4
# speedup vs baseline: 1.3875x; 1.3875x over previous
"""Trainium2 Bass kernel: gumbel-softmax-argmax embedding lookup (end-to-end).

Reference math (nn_End2End_49495203119139):
    hot  = argmax_V(softmax((logits + gumbel)/tau))  == argmax_V(logits+gumbel)
    row  = grid_sample-nearest index map of hot      == ROWMAP[hot]  (LUT)
    inputs_embeds = W[row] * mask          (col map == arange(E), verified)
    psg branch: roll/flip/rotate of psg ids, flag = cumsum(trunc!=0) > 0,
    out = inputs_embeds + where(flag, W[trunc], 0)

Key structural fact (verified exactly in numpy, dev_check.py): the attention
mask is a contiguous run of len_b ones then zeros, which makes the two
branches DISJOINT per position:
    l <  len_b: out[b,l] = W[ROWMAP[argmax_V(logits+gumbel)[b,l]]]
    l >= len_b: out[b,l] = W[psg_roll[b, l-len_b]],
                psg_roll[0] = 1 (BOS), psg_roll[j] = psg[b, j-1]
so only the sum(len_b) ACTIVE positions need the memory-bound vocab stream.
For the canonical input that is 1419/2048 positions (69%).

Sharding: the active positions are resharded EVENLY across the 8 cores
(Ta = ceil(A/8) vocab-streams per core instead of 256 batch-sharded ones),
which cuts the per-core HBM traffic from 66 MB to ~2*Ta*125.5KB + gathers.
The inactive positions are a pure indirect W-row gather (Tp per core).

Per-core layout trick: each active token's 32128-float logits row is folded
host-side to [128, 251] (partition-major), and a core's Ta tokens are
concatenated along the free axis -> DRAM [128*Ta, 251] viewed as
[128, Ta*251].  Every streaming DMA is then a full-width 128-partition
transfer regardless of Ta (a partial-partition tile would run at the same
wall time as a full one - the slowest SBUF port binds).

Device plan per core:
  - psg phase (independent of streaming, issued first): load Tp host-computed
    W-row ids, one indirect gather, store.
  - stream phase: chunks of 8 tokens = [128, 2008] f32. HWDGE(sync ring)
    loads logits chunk; SWDGE CCE-add DMA accumulates the gumbel chunk
    in the DMA datapath (one <=2048-element descriptor per partition);
    DVE reduce_max per 251-column token strip -> stats tile mx[128, 128]
    (partition p = vocab fold strip, column = token).
  - per 128-token block tail: exact DVE 32x32 stream-transposes give
    mxT[token, strip]; DVE max/max_index -> global max + winning strip p*;
    one indirect gather re-fetches the winning [1,251] strips of logits and
    gumbel (rows of the [128*Ta, 251] fold at p**Ta + t), DVE add + max_index
    -> within-strip index; hot = p**251 + c*.  ROWMAP and W rows via chained
    indirect gathers; result rows stored to out_act.  Block 0's tail is
    issued between block 0's and block 1's stream chunks so its SWDGE work
    overlaps streaming (SWDGE/HWDGE queues are FIFO - ordering matters);
    out writes go on the scalar HWDGE ring so they never stall chunk loads.

Host does only sharding/unsharding: selecting + reordering rows by the mask
(data-dependent sharding), the [128,251] fold, and scattering the returned
rows into [16,128,768]. All arithmetic on tensor VALUES happens on device.

Tie-breaking matches jnp argmax (first occurrence) exactly: vocab index =
p*251+c with strips in vocab order; DVE max_index returns the first index;
cross-partition winner is the first partition attaining the max; CCE f32
add == DVE f32 add bitwise, so streamed maxes and re-gathered strips agree.
"""

import numpy as np

B = 16
L = 128
V = 32128
E = 768
N_CORES = 8
P = 128                   # partitions; V = P * C
C = V // P                # 251 columns per folded strip
TPC = 8                   # tokens per streamed chunk -> 2008 cols <= 2048
NEG = -3.0e38


def _build(nc_mod, ta, tp):
    """Build the per-core Bass module for ta active streams + tp psg rows."""
    import concourse.tile as tile
    from concourse import bass, mybir
    from concourse.bass import IndirectOffsetOnAxis

    nc = nc_mod
    f32 = mybir.dt.float32
    i32 = mybir.dt.int32
    u32 = mybir.dt.uint32
    Op = mybir.AluOpType
    AX = mybir.AxisListType

    n_blk = (ta + P - 1) // P

    lg_h = nc.dram_tensor("lg", [P * ta, C], f32, kind="ExternalInput")
    gm_h = nc.dram_tensor("gm", [P * ta, C], f32, kind="ExternalInput")
    wte_h = nc.dram_tensor("wte", [V, E], f32, kind="ExternalInput")
    rowmap_h = nc.dram_tensor("rowmap", [V, 1], i32, kind="ExternalInput")
    out_act_h = nc.dram_tensor("out_act", [ta, E], f32, kind="ExternalOutput")
    if tp:
        psgidx_h = nc.dram_tensor("psgidx", [tp, 1], i32, kind="ExternalInput")
        out_psg_h = nc.dram_tensor("out_psg", [tp, E], f32, kind="ExternalOutput")

    # [128, ta*251] streaming views (row-major contiguous reshape)
    lg2 = lg_h[:, :].rearrange("(p t) c -> p (t c)", p=P)
    gm2 = gm_h[:, :].rearrange("(p t) c -> p (t c)", p=P)

    with tile.TileContext(nc) as tc:
        with (
            tc.tile_pool(name="lpool", bufs=8) as lpool,
            tc.tile_pool(name="stats", bufs=2 * n_blk) as stats,
            tc.tile_pool(name="small", bufs=3) as small,
            tc.tile_pool(name="emb", bufs=2) as emb,
            tc.tile_pool(name="consts", bufs=1) as consts,
        ):
            # ---- constants ----
            iota_p_i = consts.tile([P, 1], i32)
            nc.gpsimd.iota(iota_p_i[:], pattern=[[1, 1]], base=0, channel_multiplier=1)
            iota_pf = consts.tile([P, 1], f32)
            nc.vector.tensor_copy(out=iota_pf[:], in_=iota_p_i[:])
            iota_pb = []
            for b in range(n_blk):
                t = consts.tile([P, 1], f32)
                # value = b*128 + p  (token id of partition p in block b's mxT)
                nc.vector.tensor_scalar(t[:], iota_pf[:], float(b * P), None, op0=Op.add)
                iota_pb.append(t)

            # ---- psg phase: pure indirect W gather, overlaps streaming ----
            if tp:
                pidx = small.tile([tp, 1], i32, tag="pidx")
                nc.scalar.dma_start(out=pidx[:], in_=psgidx_h[:, :])
                pemb = emb.tile([tp, E], f32, tag="pemb")
                nc.gpsimd.indirect_dma_start(
                    out=pemb[:], out_offset=None, in_=wte_h[:],
                    in_offset=IndirectOffsetOnAxis(ap=pidx[:, 0:1], axis=0))
                nc.scalar.dma_start(out=out_psg_h[:, :], in_=pemb[:])

            def stream_block(b):
                """Stream block b's chunks; returns the [128, P] stats tile
                (partition = vocab strip, column = token-within-block)."""
                t0b = b * P
                nt = min(P, ta - t0b)
                mx = stats.tile([P, P], f32, tag=f"mx{b}")
                if nt < P:
                    nc.vector.memset(mx[:], NEG)
                for t0 in range(t0b, t0b + nt, TPC):
                    tn = min(TPC, t0b + nt - t0)
                    cols = tn * C
                    lt = lpool.tile([P, TPC * C], f32, tag="lt")
                    nc.sync.dma_start(out=lt[:, 0:cols],
                                      in_=lg2[:, t0 * C:(t0 + tn) * C])
                    nc.gpsimd.dma_start(out=lt[:, 0:cols],
                                        in_=gm2[:, t0 * C:(t0 + tn) * C],
                                        accum_op=Op.add)
                    for j in range(tn):
                        col = t0 - t0b + j
                        nc.vector.reduce_max(out=mx[:, col:col + 1],
                                             in_=lt[:, j * C:(j + 1) * C],
                                             axis=AX.X)
                return mx

            def tail_block(b, mx):
                """Resolve block b's argmaxes and store its embedding rows."""
                t0b = b * P
                nt = min(P, ta - t0b)
                # exact full transpose via 32x32 DVE stream-transposes
                mxT = stats.tile([P, P], f32, tag=f"mxT{b}")
                S = 32
                for i in range(P // S):
                    for j in range(P // S):
                        nc.vector.transpose(
                            out=mxT[S * j:S * j + S, S * i:S * i + S],
                            in_=mx[S * i:S * i + S, S * j:S * j + S])
                # per token: global max (col0) and first strip attaining it
                gmax8 = small.tile([P, 8], f32, tag="gmax8")
                nc.vector.max(out=gmax8[:], in_=mxT[:])
                p8 = small.tile([P, 8], u32, tag="p8")
                nc.vector.max_index(out=p8[:], in_max=gmax8[:], in_values=mxT[:])
                p1f = small.tile([P, 1], f32, tag="p1f")
                nc.vector.tensor_copy(out=p1f[:], in_=p8[:, 0:1])
                # fold row of the winning strip: p* * ta + token_id
                rowsf = small.tile([P, 1], f32, tag="rowsf")
                nc.vector.scalar_tensor_tensor(
                    out=rowsf[:], in0=p1f[:], scalar=float(ta), in1=iota_pb[b][:],
                    op0=Op.mult, op1=Op.add)
                rows_i = small.tile([P, 1], i32, tag="rows_i")
                nc.vector.tensor_copy(out=rows_i[:], in_=rowsf[:])
                # re-fetch the winning strips, recompute l+g exactly
                stl = emb.tile([P, C], f32, tag="stl")
                nc.gpsimd.indirect_dma_start(
                    out=stl[:], out_offset=None, in_=lg_h[:],
                    in_offset=IndirectOffsetOnAxis(ap=rows_i[:, 0:1], axis=0))
                stg = emb.tile([P, C], f32, tag="stg")
                nc.gpsimd.indirect_dma_start(
                    out=stg[:], out_offset=None, in_=gm_h[:],
                    in_offset=IndirectOffsetOnAxis(ap=rows_i[:, 0:1], axis=0))
                nc.vector.tensor_tensor(out=stl[:], in0=stl[:], in1=stg[:], op=Op.add)
                s8 = small.tile([P, 8], f32, tag="s8")
                nc.vector.max(out=s8[:], in_=stl[:])
                c8 = small.tile([P, 8], u32, tag="c8")
                nc.vector.max_index(out=c8[:], in_max=s8[:], in_values=stl[:])
                c1f = small.tile([P, 1], f32, tag="c1f")
                nc.vector.tensor_copy(out=c1f[:], in_=c8[:, 0:1])
                # hot vocab index = p* * 251 + c*
                hotf = small.tile([P, 1], f32, tag="hotf")
                nc.vector.scalar_tensor_tensor(
                    out=hotf[:], in0=p1f[:], scalar=float(C), in1=c1f[:],
                    op0=Op.mult, op1=Op.add)
                hot_i = small.tile([P, 1], i32, tag="hot_i")
                nc.vector.tensor_copy(out=hot_i[:], in_=hotf[:])
                # grid_sample-nearest LUT, then the embedding rows
                rowv = small.tile([P, 1], i32, tag="rowv")
                nc.gpsimd.indirect_dma_start(
                    out=rowv[:], out_offset=None, in_=rowmap_h[:],
                    in_offset=IndirectOffsetOnAxis(ap=hot_i[:, 0:1], axis=0))
                wrows = emb.tile([P, E], f32, tag="wrows")
                nc.gpsimd.indirect_dma_start(
                    out=wrows[:], out_offset=None, in_=wte_h[:],
                    in_offset=IndirectOffsetOnAxis(ap=rowv[:, 0:1], axis=0))
                # scalar ring: never stalls the chunk loads on the sync ring
                nc.scalar.dma_start(out=out_act_h[t0b:t0b + nt, :],
                                    in_=wrows[0:nt, :])

            # tail of block b-1 is issued between block b-1's and block b's
            # chunks: its SWDGE gathers overlap block b's streaming.
            prev = None
            for b in range(n_blk):
                mx = stream_block(b)
                if prev is not None:
                    tail_block(b - 1, prev)
                prev = mx
            tail_block(n_blk - 1, prev)

    return nc


_BUILD_CACHE = {}


def _get_module(ta, tp):
    key = (ta, tp)
    if key not in _BUILD_CACHE:
        import concourse.bacc as bacc

        nc = bacc.Bacc("TRN2", target_bir_lowering=False, debug=False)
        _build(nc, ta, tp)
        nc.compile()
        _BUILD_CACHE[key] = nc
    return _BUILD_CACHE[key]


# The reference's f32 grid_sample-nearest index maps, precomputed with jnp
# (the backend the reference runs on) for the hardcoded V=32128 / E=768:
# the column map is exactly identity; the row map is identity except at
# these 17 indices (f32 rounding of the normalized-coordinate roundtrip).
_ROWMAP_DIFF_IDX = [1, 2, 6, 11, 16, 32079, 32089, 32093, 32099, 32103,
                    32107, 32109, 32113, 32117, 32119, 32121, 32123]
_ROWMAP_DIFF_VAL = [0, 1, 5, 10, 15, 32080, 32090, 32094, 32100, 32104,
                    32108, 32110, 32114, 32118, 32120, 32122, 32124]


def _nearest_maps():
    rowmap = np.arange(V, dtype=np.int32)
    rowmap[_ROWMAP_DIFF_IDX] = _ROWMAP_DIFF_VAL
    return rowmap, np.arange(E, dtype=np.int32)


# test/dev hooks: set TRACE=True before calling kernel() to capture an NTFF
# profile; the BassKernelResults of the last run is stored in LAST_RESULT.
TRACE = False
LAST_RESULT = None
LAST_MODULE = None


def _fold(rows):
    """[n, V] f32 -> [128*n, 251] partition-major fold."""
    n = rows.shape[0]
    return np.ascontiguousarray(
        rows.reshape(n, P, C).transpose(1, 0, 2).reshape(P * n, C))


def kernel(logits, rwrt_attention_mask, psg_input_ids, word_embeddings, gumbel_noise):
    from concourse.bass_utils import run_bass_kernel_spmd

    logits = np.ascontiguousarray(np.asarray(logits, dtype=np.float32)).reshape(B * L, V)
    gumbel = np.ascontiguousarray(np.asarray(gumbel_noise, dtype=np.float32)).reshape(B * L, V)
    mask = np.asarray(rwrt_attention_mask, dtype=np.int32)
    psg = np.asarray(psg_input_ids, dtype=np.int32)
    wte = np.ascontiguousarray(np.asarray(word_embeddings, dtype=np.float32))

    rowmap, colmap = _nearest_maps()
    assert np.array_equal(colmap, np.arange(E, dtype=np.int32)), "col map not identity"
    rowmap2 = np.ascontiguousarray(rowmap.reshape(V, 1))

    lens = mask.sum(axis=1)
    contiguous = bool(np.all(mask == (np.arange(L)[None, :] < lens[:, None])))
    assert contiguous, "non-contiguous attention mask unsupported by this kernel"

    # active positions (vocab-stream needed) and psg positions (W-row gather)
    act_pos = []           # flat b*L+l, in output order
    psg_pos = []
    psg_rows = []
    for b in range(B):
        ln = int(lens[b])
        act_pos.extend(b * L + l for l in range(ln))
        for l in range(ln, L):
            psg_pos.append(b * L + l)
            psg_rows.append(1 if l == ln else int(psg[b, l - ln - 1]))
    A, Pn = len(act_pos), len(psg_pos)
    ta = max(1, (A + N_CORES - 1) // N_CORES)
    tp = (Pn + N_CORES - 1) // N_CORES

    act_idx = np.asarray(act_pos + [act_pos[-1]] * (ta * N_CORES - A), np.int64)
    psg_idx = np.asarray(psg_rows + [0] * (tp * N_CORES - Pn), np.int32)

    nc = _get_module(ta, tp)
    global LAST_MODULE
    LAST_MODULE = nc

    in_maps = []
    for m in range(N_CORES):
        sl = act_idx[m * ta:(m + 1) * ta]
        im = {
            "lg": _fold(logits[sl]),
            "gm": _fold(gumbel[sl]),
            "wte": wte,
            "rowmap": rowmap2,
        }
        if tp:
            im["psgidx"] = np.ascontiguousarray(
                psg_idx[m * tp:(m + 1) * tp].reshape(tp, 1))
        in_maps.append(im)

    global LAST_RESULT
    try:
        LAST_RESULT = run_bass_kernel_spmd(nc, in_maps, list(range(N_CORES)), trace=TRACE)
    except Exception:
        # the axon-relayed device occasionally reports a transient
        # NRT_EXEC_UNIT_UNRECOVERABLE on the first execution after long
        # sessions; a straight re-run recovers it
        import time as _time

        _time.sleep(2.0)
        LAST_RESULT = run_bass_kernel_spmd(nc, in_maps, list(range(N_CORES)), trace=TRACE)
    res = LAST_RESULT.results

    out = np.empty((B * L, E), np.float32)
    acts = np.concatenate([res[m]["out_act"] for m in range(N_CORES)], axis=0)
    out[np.asarray(act_pos, np.int64)] = acts[:A]
    if Pn:
        psgs = np.concatenate([res[m]["out_psg"] for m in range(N_CORES)], axis=0)
        out[np.asarray(psg_pos, np.int64)] = psgs[:Pn]
    return out.reshape(B, L, E)


# revision 47
# speedup vs baseline: 1.4254x; 1.0273x over previous
"""Trainium2 Bass kernel: gumbel-softmax-argmax embedding lookup (end-to-end).

Reference math (nn_End2End_49495203119139):
    hot  = argmax_V(softmax((logits + gumbel)/tau))  == argmax_V(logits+gumbel)
    row  = grid_sample-nearest index map of hot      == ROWMAP[hot]  (LUT)
    inputs_embeds = W[row] * mask          (col map == arange(E), verified)
    psg branch: roll/flip/rotate of psg ids, flag = cumsum(trunc!=0) > 0,
    out = inputs_embeds + where(flag, W[trunc], 0)

Key structural fact (verified exactly in numpy, dev_check.py): the attention
mask is a contiguous run of len_b ones then zeros, which makes the two
branches DISJOINT per position:
    l <  len_b: out[b,l] = W[ROWMAP[argmax_V(logits+gumbel)[b,l]]]
    l >= len_b: out[b,l] = W[psg_roll[b, l-len_b]],
                psg_roll[0] = 1 (BOS), psg_roll[j] = psg[b, j-1]
so only the sum(len_b) ACTIVE positions need the memory-bound vocab stream.
For the canonical input that is 1419/2048 positions (69%).

Sharding: the active positions are resharded EVENLY across the 8 cores
(Ta = ceil(A/8) vocab-streams per core instead of 256 batch-sharded ones),
which cuts the per-core HBM traffic from 66 MB to ~2*Ta*125.5KB + gathers.
The inactive positions are a pure indirect W-row gather (Tp per core).

Per-core layout trick: each active token's 32128-float logits row is folded
host-side to [128, 251] (partition-major), and a core's Ta tokens are
concatenated along the free axis -> DRAM [128*Ta, 251] viewed as
[128, Ta*251].  Every streaming DMA is then a full-width 128-partition
transfer regardless of Ta (a partial-partition tile would run at the same
wall time as a full one - the slowest SBUF port binds).

Device plan per core:
  - psg phase (independent of streaming, issued first): load Tp host-computed
    W-row ids, one indirect gather, store.
  - stream phase: chunks of 8 tokens = [128, 2008] f32. HWDGE(sync ring)
    loads logits chunk; SWDGE CCE-add DMA accumulates the gumbel chunk
    in the DMA datapath (one <=2048-element descriptor per partition);
    DVE reduce_max per 251-column token strip -> stats tile mx[128, 128]
    (partition p = vocab fold strip, column = token).
  - per 128-token block tail: exact DVE 32x32 stream-transposes give
    mxT[token, strip]; DVE max/max_index -> global max + winning strip p*;
    one indirect gather re-fetches the winning [1,251] strips of logits and
    gumbel (rows of the [128*Ta, 251] fold at p**Ta + t), DVE add + max_index
    -> within-strip index; hot = p**251 + c*.  ROWMAP and W rows via chained
    indirect gathers; result rows stored to out_act.  Block 0's tail is
    issued between block 0's and block 1's stream chunks so its SWDGE work
    overlaps streaming (SWDGE/HWDGE queues are FIFO - ordering matters);
    out writes go on the scalar HWDGE ring so they never stall chunk loads.

Host does only sharding/unsharding: selecting + reordering rows by the mask
(data-dependent sharding), the [128,251] fold, and scattering the returned
rows into [16,128,768]. All arithmetic on tensor VALUES happens on device.

Tie-breaking matches jnp argmax (first occurrence) exactly: vocab index =
p*251+c with strips in vocab order; DVE max_index returns the first index;
cross-partition winner is the first partition attaining the max; CCE f32
add == DVE f32 add bitwise, so streamed maxes and re-gathered strips agree.
"""

import numpy as np

B = 16
L = 128
V = 32128
E = 768
N_CORES = 8
P = 128                   # partitions; V = P * C
C = V // P                # 251 columns per folded strip
TPC = 8                   # tokens per streamed chunk -> 2008 cols <= 2048
NEG = -3.0e38


def _build(nc_mod, ta, tp, variant=None):
    """Build the per-core Bass module for ta active streams + tp psg rows."""
    import concourse.tile as tile
    from concourse import bass, mybir
    from concourse.bass import IndirectOffsetOnAxis

    var = variant or {}

    nc = nc_mod
    f32 = mybir.dt.float32
    i32 = mybir.dt.int32
    u32 = mybir.dt.uint32
    Op = mybir.AluOpType
    AX = mybir.AxisListType

    n_blk = (ta + P - 1) // P

    # logits fold rows [0, P*ta), gumbel fold rows [P*ta, 2*P*ta): one tensor
    # so a single 2-index indirect gather fetches both winning strips
    lgg_h = nc.dram_tensor("lgg", [2 * P * ta, C], f32, kind="ExternalInput")
    # wrm = W[rowmap] precomposed host-side (rowmap is a pure LUT of V)
    wrm_h = nc.dram_tensor("wrm", [V, E], f32, kind="ExternalInput")
    out_act_h = nc.dram_tensor("out_act", [ta, E], f32, kind="ExternalOutput")
    if tp:
        wte_h = nc.dram_tensor("wte", [V, E], f32, kind="ExternalInput")
        psgidx_h = nc.dram_tensor("psgidx", [tp, 1], i32, kind="ExternalInput")
        out_psg_h = nc.dram_tensor("out_psg", [tp, E], f32, kind="ExternalOutput")

    # [128, ta*251] streaming views (row-major contiguous reshapes)
    lg2 = lgg_h[0:P * ta, :].rearrange("(p t) c -> p (t c)", p=P)
    gm2 = lgg_h[P * ta:2 * P * ta, :].rearrange("(p t) c -> p (t c)", p=P)

    with tile.TileContext(nc) as tc:
        with (
            tc.tile_pool(name="lpool", bufs=var.get("LBUFS", 8)) as lpool,
            tc.tile_pool(name="stats", bufs=4 * n_blk) as stats,
            tc.tile_pool(name="small", bufs=3) as small,
            tc.tile_pool(name="emb", bufs=2) as emb,
            tc.tile_pool(name="consts", bufs=1) as consts,
        ):
            # ---- constants ----
            iota_p_i = consts.tile([P, 1], i32)
            nc.gpsimd.iota(iota_p_i[:], pattern=[[1, 1]], base=0, channel_multiplier=1)
            iota_pf = consts.tile([P, 1], f32)
            nc.vector.tensor_copy(out=iota_pf[:], in_=iota_p_i[:])
            idx_mode = var.get("IDX_MODE", False)
            if idx_mode:
                # per-partition row 0..127 along the free axis (strip ids)
                iota_c_i = consts.tile([P, P], i32)
                nc.gpsimd.iota(iota_c_i[:], pattern=[[1, P]], base=0,
                               channel_multiplier=0)
                iota_cols = consts.tile([P, P], f32)
                nc.vector.tensor_copy(out=iota_cols[:], in_=iota_c_i[:])

            # ---- psg phase: pure indirect W gather, overlaps streaming ----
            if tp:
                pidx = small.tile([tp, 1], i32, tag="pidx")
                nc.scalar.dma_start(out=pidx[:], in_=psgidx_h[:, :])
                pemb = emb.tile([tp, E], f32, tag="pemb")
                nc.gpsimd.indirect_dma_start(
                    out=pemb[:], out_offset=None, in_=wte_h[:],
                    in_offset=IndirectOffsetOnAxis(ap=pidx[:, 0:1], axis=0))
                nc.scalar.dma_start(out=out_psg_h[:, :], in_=pemb[:])

            S = 32                            # transpose block size
            GS = var.get("GS", 64)            # granule: tokens per tail set

            def granule_phases(b, g, mx, ix=None):
                """Phases resolving tokens [b*128+GS*g, +GS): each later
                phase's work depends only on phases issued >= one chunk
                earlier, so the in-order SWDGE/DVE queues never stall the
                streaming on tail dependencies."""
                t0b = b * P
                nt = min(P, ta - t0b)
                lo = g * GS                      # local token range [lo, hi)
                hi = min(lo + GS, nt)
                gs = hi - lo                     # real tokens (<= GS)
                gsp = min(GS, P - lo)            # padded partition extent
                st = {}

                def ph0():
                    # transpose mx[:, lo:lo+gsp] into a base-0 token-major
                    # tile (exact 32x32 copies) and resolve p* per token.
                    # All compute tiles sit at partition base 0: the BIR
                    # verifier requires equal base partitions for two-SB-input
                    # instructions (NCC_IBIR297).
                    mxTg = stats.tile([gsp, P], f32, tag="mxTg")
                    for j in range(gsp // S):
                        for i in range(P // S):
                            nc.vector.transpose(
                                out=mxTg[S * j:S * (j + 1), S * i:S * i + S],
                                in_=mx[S * i:S * i + S,
                                       lo + S * j:lo + S * (j + 1)])
                    if idx_mode:
                        ixTg = stats.tile([gsp, P], u32, tag="ixTg")
                        for j in range(gsp // S):
                            for i in range(P // S):
                                nc.vector.transpose(
                                    out=ixTg[S * j:S * (j + 1), S * i:S * i + S],
                                    in_=ix[S * i:S * i + S,
                                           lo + S * j:lo + S * (j + 1)])
                        st["ixTg"] = ixTg
                    gmax8 = small.tile([gsp, 8], f32, tag="gmax8")
                    nc.vector.max(out=gmax8[:], in_=mxTg[:])
                    p8 = small.tile([gsp, 8], u32, tag="p8")
                    nc.vector.max_index(out=p8[:], in_max=gmax8[:], in_values=mxTg[:])
                    p1f = small.tile([gsp, 1], f32, tag="p1f")
                    nc.vector.tensor_copy(out=p1f[:], in_=p8[:, 0:1])
                    # token ids of this granule, at partition base 0
                    tofs = small.tile([gsp, 1], f32, tag="tofs")
                    nc.vector.tensor_scalar(tofs[:], iota_pf[0:gsp],
                                            float(b * P + lo), None, op0=Op.add)
                    # fold rows of the winning strips in lgg: col0 = logits
                    # half (p*ta + t), col1 = gumbel half (+ P*ta)
                    rows2 = small.tile([gsp, 2], f32, tag="rows2")
                    nc.vector.scalar_tensor_tensor(
                        out=rows2[:, 0:1], in0=p1f[:], scalar=float(ta),
                        in1=tofs[:], op0=Op.mult, op1=Op.add)
                    nc.vector.tensor_scalar(rows2[:, 1:2], rows2[:, 0:1],
                                            float(P * ta), None, op0=Op.add)
                    rows2i = small.tile([gsp, 2], i32, tag="rows2i")
                    nc.vector.tensor_copy(out=rows2i[:], in_=rows2[:])
                    st["p1f"], st["rows2i"] = p1f, rows2i

                def ph0_idx():
                    # c* from the in-stream index stats: select column p* of
                    # ixTg (one masked multiply + reduce), no strip re-fetch
                    ixTf = small.tile([gsp, P], f32, tag="ixTf")
                    nc.vector.tensor_copy(out=ixTf[:], in_=st["ixTg"][:])
                    selx = small.tile([gsp, P], f32, tag="selx")
                    nc.vector.scalar_tensor_tensor(
                        out=selx[:], in0=iota_cols[0:gsp, :],
                        scalar=st["p1f"][:, 0:1], in1=ixTf[:],
                        op0=Op.is_equal, op1=Op.mult)
                    c1f = small.tile([gsp, 1], f32, tag="c1f")
                    nc.vector.reduce_max(out=c1f[:], in_=selx[:], axis=AX.X)
                    hotf = small.tile([gsp, 1], f32, tag="hotf")
                    nc.vector.scalar_tensor_tensor(
                        out=hotf[:], in0=st["p1f"], scalar=float(C), in1=c1f[:],
                        op0=Op.mult, op1=Op.add)
                    hot_i = small.tile([gsp, 1], i32, tag="hot_i")
                    nc.vector.tensor_copy(out=hot_i[:], in_=hotf[:])
                    st["hot_i"] = hot_i

                def ph1a():
                    # two single-index gathers fetch the winning logits and
                    # gumbel strips. (A fused 2-index gather - ap=[:,0:2],
                    # out [gsp, 2C] - simulates correctly in CoreSim but
                    # returns wrong rows on hardware; keep them separate.)
                    stl = emb.tile([gsp, 2 * C], f32, tag="stl")
                    nc.gpsimd.indirect_dma_start(
                        out=stl[:, 0:C], out_offset=None, in_=lgg_h[:],
                        in_offset=IndirectOffsetOnAxis(
                            ap=st["rows2i"][:, 0:1], axis=0))
                    nc.gpsimd.indirect_dma_start(
                        out=stl[:, C:2 * C], out_offset=None, in_=lgg_h[:],
                        in_offset=IndirectOffsetOnAxis(
                            ap=st["rows2i"][:, 1:2], axis=0))
                    st["stl"] = stl

                def ph1b():
                    # recompute l+g on the fetched strips, find c*
                    stl = st["stl"]
                    ssum = emb.tile([gsp, C], f32, tag="ssum")
                    nc.vector.tensor_tensor(out=ssum[:], in0=stl[:, 0:C],
                                            in1=stl[:, C:2 * C], op=Op.add)
                    s8 = small.tile([gsp, 8], f32, tag="s8")
                    nc.vector.max(out=s8[:], in_=ssum[:])
                    c8 = small.tile([gsp, 8], u32, tag="c8")
                    nc.vector.max_index(out=c8[:], in_max=s8[:], in_values=ssum[:])
                    c1f = small.tile([gsp, 1], f32, tag="c1f")
                    nc.vector.tensor_copy(out=c1f[:], in_=c8[:, 0:1])
                    hotf = small.tile([gsp, 1], f32, tag="hotf")
                    nc.vector.scalar_tensor_tensor(
                        out=hotf[:], in0=st["p1f"], scalar=float(C), in1=c1f[:],
                        op0=Op.mult, op1=Op.add)
                    hot_i = small.tile([gsp, 1], i32, tag="hot_i")
                    nc.vector.tensor_copy(out=hot_i[:], in_=hotf[:])
                    st["hot_i"] = hot_i

                def ph2():
                    # W[rowmap[.]] is precomposed host-side into wrm
                    wrows = emb.tile([gsp, E], f32, tag="wrows")
                    nc.gpsimd.indirect_dma_start(
                        out=wrows[:], out_offset=None, in_=wrm_h[:],
                        in_offset=IndirectOffsetOnAxis(ap=st["hot_i"][:, 0:1],
                                                       axis=0))
                    st["wrows"] = wrows

                def ph3():
                    # scalar ring: never stalls the chunk loads on sync
                    nc.scalar.dma_start(out=out_act_h[t0b + lo:t0b + hi, :],
                                        in_=st["wrows"][0:gs, :])

                if idx_mode:
                    phases = [ph0, ph0_idx, ph2, ph3]
                else:
                    phases = [ph0, ph1a, ph1b, ph2, ph3]
                return phases[:var.get("TAIL_LEVEL", 9)]

            # chunk schedule across all blocks, then interleave tail phases
            tpc_v = var.get("TPC", TPC)
            chunks = []          # (b, t0, tn)
            for b in range(n_blk):
                t0b = b * P
                nt = min(P, ta - t0b)
                for t0 in range(t0b, t0b + nt, tpc_v):
                    tn = min(tpc_v, t0b + nt - t0)
                    chunks.append((b, t0, tn))

            blk_tiles = {}
            pending = []         # (due_chunk_idx, phase_fn)
            gran_seen = set()
            for ci, (b, t0, tn) in enumerate(chunks):
                t0b = b * P
                nt = min(P, ta - t0b)
                if b not in blk_tiles:
                    mx = stats.tile([P, P], f32, tag=f"mx{b}")
                    ix = None
                    if idx_mode:
                        ix = stats.tile([P, P], u32, tag=f"ix{b}")
                    if nt < P:
                        nc.vector.memset(mx[:], NEG)
                        if idx_mode:
                            nc.vector.memset(ix[:], 0)
                    blk_tiles[b] = (mx, ix)
                mx, ix = blk_tiles[b]
                cols = tn * C
                tpc = var.get("TPC", TPC)
                lt = lpool.tile([P, tpc * C], f32, tag="lt")
                ldeng = nc.scalar if (var.get("DUAL_HWDGE") and ci % 2) else nc.sync
                ldeng.dma_start(out=lt[:, 0:cols],
                                in_=lg2[:, t0 * C:(t0 + tn) * C])
                if not var.get("SKIP_ACCUM"):
                    # CCE-add descriptors must stay <= 2048 elements on HW:
                    # split the accumulate at 8-token granularity
                    for a0 in range(0, tn, TPC):
                        an = min(TPC, tn - a0)
                        nc.gpsimd.dma_start(
                            out=lt[:, a0 * C:(a0 + an) * C],
                            in_=gm2[:, (t0 + a0) * C:(t0 + a0 + an) * C],
                            accum_op=Op.add)
                if not var.get("SKIP_REDUCE"):
                    col = t0 - t0b
                    if var.get("UNBATCH_REDUCE"):
                        for j in range(tn):
                            nc.vector.reduce_max(out=mx[:, col + j:col + j + 1],
                                                 in_=lt[:, j * C:(j + 1) * C],
                                                 axis=AX.X)
                    else:
                        # all tn per-token strip maxes in ONE 3D-AP reduce
                        nc.vector.reduce_max(
                            out=mx[:, col:col + tn],
                            in_=lt[:, 0:cols].rearrange("p (t c) -> p t c", c=C),
                            axis=AX.X)
                    if idx_mode:
                        # within-strip argmax per token, tracked in-stream
                        for j in range(tn):
                            ix8 = small.tile([P, 8], u32, tag="ix8")
                            nc.vector.max_index(
                                out=ix8[:],
                                in_max=mx[:, col + j:col + j + 1].to_broadcast([P, 8]),
                                in_values=lt[:, j * C:(j + 1) * C])
                            nc.scalar.copy(out=ix[:, col + j:col + j + 1],
                                           in_=ix8[:, 0:1])
                elif t0 + tn >= t0b + nt:
                    nc.vector.reduce_max(out=mx[:, 0:1],
                                         in_=lt[:, 0:C], axis=AX.X)
                if var.get("SKIP_TAILS"):
                    continue
                # queue tail phases for granules completed by this chunk
                streamed = t0 - t0b + tn
                for g in range((nt + GS - 1) // GS):
                    if (b, g) in gran_seen:
                        continue
                    if streamed >= min((g + 1) * GS, nt):
                        gran_seen.add((b, g))
                        dues = var.get("PH_DUES",
                                       (0, 1, 2, 3) if idx_mode else (0, 1, 2, 3, 4))
                        phs = granule_phases(b, g, mx, ix)
                        for k, ph in enumerate(phs):
                            pending.append((ci + dues[k], ph))
                # emit everything due after this chunk, in phase order
                due = [x for x in pending if x[0] <= ci]
                pending = [x for x in pending if x[0] > ci]
                for _, ph in due:
                    ph()
            for _, ph in sorted(pending, key=lambda x: x[0]):
                ph()

    return nc


_BUILD_CACHE = {}


def _get_module(ta, tp, variant=None):
    key = (ta, tp, tuple(sorted((variant or {}).items())))
    if key not in _BUILD_CACHE:
        import concourse.bacc as bacc

        nc = bacc.Bacc("TRN2", target_bir_lowering=False, debug=False)
        _build(nc, ta, tp, variant)
        nc.compile()
        _BUILD_CACHE[key] = nc
    return _BUILD_CACHE[key]


# The reference's f32 grid_sample-nearest index maps, precomputed with jnp
# (the backend the reference runs on) for the hardcoded V=32128 / E=768:
# the column map is exactly identity; the row map is identity except at
# these 17 indices (f32 rounding of the normalized-coordinate roundtrip).
_ROWMAP_DIFF_IDX = [1, 2, 6, 11, 16, 32079, 32089, 32093, 32099, 32103,
                    32107, 32109, 32113, 32117, 32119, 32121, 32123]
_ROWMAP_DIFF_VAL = [0, 1, 5, 10, 15, 32080, 32090, 32094, 32100, 32104,
                    32108, 32110, 32114, 32118, 32120, 32122, 32124]


def _nearest_maps():
    rowmap = np.arange(V, dtype=np.int32)
    rowmap[_ROWMAP_DIFF_IDX] = _ROWMAP_DIFF_VAL
    return rowmap, np.arange(E, dtype=np.int32)


# test/dev hooks: set TRACE=True before calling kernel() to capture an NTFF
# profile; the BassKernelResults of the last run is stored in LAST_RESULT.
TRACE = False
LAST_RESULT = None
LAST_MODULE = None
DEFAULT_VARIANT = None   # dev hook: build-variant dict used by kernel()


def _fold(rows):
    """[n, V] f32 -> [128*n, 251] partition-major fold."""
    n = rows.shape[0]
    return np.ascontiguousarray(
        rows.reshape(n, P, C).transpose(1, 0, 2).reshape(P * n, C))


def kernel(logits, rwrt_attention_mask, psg_input_ids, word_embeddings, gumbel_noise):
    from concourse.bass_utils import run_bass_kernel_spmd

    logits = np.ascontiguousarray(np.asarray(logits, dtype=np.float32)).reshape(B * L, V)
    gumbel = np.ascontiguousarray(np.asarray(gumbel_noise, dtype=np.float32)).reshape(B * L, V)
    mask = np.asarray(rwrt_attention_mask, dtype=np.int32)
    psg = np.asarray(psg_input_ids, dtype=np.int32)
    wte = np.ascontiguousarray(np.asarray(word_embeddings, dtype=np.float32))

    # wrm = W[rowmap] precomposed (rowmap is identity except 17 rows)
    wrm = wte.copy()
    wrm[_ROWMAP_DIFF_IDX] = wte[_ROWMAP_DIFF_VAL]

    lens = mask.sum(axis=1)
    contiguous = bool(np.all(mask == (np.arange(L)[None, :] < lens[:, None])))

    if contiguous:
        # fast path: the two branches are positionally disjoint (see header)
        act_pos = []           # flat b*L+l, in output order
        psg_pos = []
        psg_rows = []
        for b in range(B):
            ln = int(lens[b])
            act_pos.extend(b * L + l for l in range(ln))
            for l in range(ln, L):
                psg_pos.append(b * L + l)
                psg_rows.append(1 if l == ln else int(psg[b, l - ln - 1]))
    else:
        # general fallback (never taken for the reference's inputs): stream
        # every position's argmax on device, gather both branches' W rows on
        # device, combine per the reference's mask/flag weights at unshard
        # time. Index arithmetic below mirrors the reference exactly.
        act_pos = list(range(B * L))
        psg_roll = np.roll(psg, 1, axis=1)
        psg_roll[:, 0] = 1
        extr = (1 - mask[:, ::-1]) * psg_roll
        pos = (np.arange(L)[None, :] - lens[:, None]) % L
        trunc = np.take_along_axis(extr, pos, axis=1)
        flag = (np.cumsum(trunc != 0, axis=1) > 0).astype(np.float32)
        psg_pos = list(range(B * L))
        psg_rows = trunc.reshape(-1).tolist()
    A, Pn = len(act_pos), len(psg_pos)
    ta = max(1, (A + N_CORES - 1) // N_CORES)
    tp = (Pn + N_CORES - 1) // N_CORES

    act_idx = np.asarray(act_pos + [act_pos[-1]] * (ta * N_CORES - A), np.int64)
    psg_idx = np.asarray(psg_rows + [0] * (tp * N_CORES - Pn), np.int32)

    nc = _get_module(ta, tp, DEFAULT_VARIANT)
    global LAST_MODULE
    LAST_MODULE = nc

    in_maps = []
    for m in range(N_CORES):
        sl = act_idx[m * ta:(m + 1) * ta]
        im = {
            "lgg": np.concatenate([_fold(logits[sl]), _fold(gumbel[sl])], axis=0),
            "wrm": wrm,
        }
        if tp:
            im["wte"] = wte
            im["psgidx"] = np.ascontiguousarray(
                psg_idx[m * tp:(m + 1) * tp].reshape(tp, 1))
        in_maps.append(im)

    global LAST_RESULT
    try:
        LAST_RESULT = run_bass_kernel_spmd(nc, in_maps, list(range(N_CORES)), trace=TRACE)
    except Exception:
        # the axon-relayed device occasionally reports a transient
        # NRT_EXEC_UNIT_UNRECOVERABLE on the first execution after long
        # sessions; a straight re-run recovers it
        import time as _time

        _time.sleep(2.0)
        LAST_RESULT = run_bass_kernel_spmd(nc, in_maps, list(range(N_CORES)), trace=TRACE)
    res = LAST_RESULT.results

    acts = np.concatenate([res[m]["out_act"] for m in range(N_CORES)], axis=0)
    if contiguous:
        out = np.empty((B * L, E), np.float32)
        out[np.asarray(act_pos, np.int64)] = acts[:A]
        if Pn:
            psgs = np.concatenate(
                [res[m]["out_psg"] for m in range(N_CORES)], axis=0)
            out[np.asarray(psg_pos, np.int64)] = psgs[:Pn]
        return out.reshape(B, L, E)
    psgs = np.concatenate([res[m]["out_psg"] for m in range(N_CORES)], axis=0)
    out = (acts[:A] * mask.reshape(-1, 1)
           + psgs[:Pn] * flag.reshape(-1, 1)).astype(np.float32)
    return out.reshape(B, L, E)
